# revision 39
# baseline (speedup 1.0000x reference)
"""Multi-head attention (B=16, N=1024, E=768, H=12) on 8 TRN2 NeuronCores.

Data parallel over batch (2 per core, no collectives). Per-core fused kernel
(fp8-DoubleRow Q/K projection + fused single-stream schedule):

  - Q/K projection in fp8-e4m3 DoubleRow (contraction 256 per matmul, so 3
    accumulation passes instead of 6): host passes x^T and W_qkv[:, :2E]
    pre-interleaved as [128, cc, i, n] with e = 256*cc + 128*i + p.
  - K-bias is dropped entirely: (q+bq)@(k+bk) = (q+bq)@k + f(q), and per-q
    terms cancel in softmax (exact).  Q gets its bias during the PSUM drain.
  - Q/K tiles stored fp8 feature-major (128 partitions = two heads' d-rows),
    12 tiles of (128, T); energy per head pair = two fp8 K=64 matmuls at PE
    row offsets 0/64 (row-tiled, partially concurrent) into one (128,1024)
    PSUM tile; one Exp ACTIVATE drains it (scale=1/8, no max subtraction -
    |logit| < 2 by construction).
  - attn@V in bf16: lhsT = [V | 1] (M=65); PSUM row 64 = softmax denom;
    both halves of a group accumulate into one (65,1024) PSUM tile so the
    denominator copy/reciprocal run once per group at (1,1024).
  - Normalization: reciprocal_approx_fast + gpsimd partition broadcast,
    fused into the PSUM->SBUF multiply that writes the shifted-duplicate
    "DOUBLE" layout; stride-12 APs over DOUBLE give exact 128-row slabs of
    Y^T for the reference's scrambled (H,N,D)->(N,E) reshape, so the out
    projection is 6 clean K=128 accumulating matmuls per 128-token tile.
  - Single fused schedule, batch-major: V-proj for batch 1, residual QK
    projection groups, and out-projection chunks drip through the attention
    loop's PE slack (borrowing PSUM slots); out-proj chunk npc only needs
    the scrambled m-prefix 1536*(npc+1)+1, so chunks start after ~1.5 heads
    of attention and only ~2 chunks remain after the last drain.
"""

import contextlib

import numpy as np

import concourse.bass as bass
import concourse.tile as tile
import concourse.mybir as mybir
from concourse import bacc
from concourse import bass_utils

B, N, E, H = 16, 1024, 768, 12
D = E // H          # 64
N_CORES = 8
BPC = B // N_CORES  # 2
T = BPC * N         # 2048
SCALE = 1.0 / float(np.sqrt(np.float32(D)))

FP32 = mybir.dt.float32
BF16 = mybir.dt.bfloat16
FP8 = mybir.dt.float8e4
AF = mybir.ActivationFunctionType
OP = mybir.AluOpType
DR = mybir.MatmulPerfMode.DoubleRow

EC = E // 128       # 6
TC16 = T // 128     # 16
HM = H * N          # 12288


def _emit(tc, x8_ap, w8_ap, xt_ap, wv_ap, bqkv_ap, wout_ap, bout_ap, out_ap):
    nc = tc.nc

    stack = contextlib.ExitStack()
    with stack:
        const_pool = stack.enter_context(tc.tile_pool(name="const", bufs=1))
        w_pool = stack.enter_context(tc.tile_pool(name="w", bufs=1))
        qk8_pool = stack.enter_context(tc.tile_pool(name="qk8", bufs=1))
        vo_pool = stack.enter_context(tc.tile_pool(name="vo", bufs=1))
        dbl_pool = stack.enter_context(tc.tile_pool(name="dbl", bufs=1))

        pse = stack.enter_context(
            tc.tile_pool(name="pse", bufs=2, space="PSUM"))   # (128,1024) = 2 banks
        pso = stack.enter_context(
            tc.tile_pool(name="pso", bufs=2, space="PSUM"))   # (65,1024) = 2 banks

        # ---- big input DMAs first (w8/x8 gate the first psum groups).
        # x8 is sliced by TOKEN half (strided over the i slots) so the
        # batch-0 projection's dependencies land first.
        x8 = w_pool.tile([128, 6 * T], FP8, tag="x8")
        w8 = w_pool.tile([128, 9216], FP8, tag="w8")
        for cc in range(3):
            nc.gpsimd.dma_start(w8[:, cc * 3072:(cc + 1) * 3072],
                                w8_ap[:, cc * 3072:(cc + 1) * 3072])
        for th in range(2):     # token half: 0:1024 first (batch 0)
            for cc in range(3):
                dst = x8[:, cc * 2 * T:(cc + 1) * 2 * T].rearrange(
                    "p (i n) -> p i n", i=2)[:, :, th * 1024:(th + 1) * 1024]
                src = x8_ap[:, cc * 2 * T:(cc + 1) * 2 * T].rearrange(
                    "p (i n) -> p i n", i=2)[:, :, th * 1024:(th + 1) * 1024]
                nc.sync.dma_start(dst, src)
        xt = [w_pool.tile([128, T], BF16, tag=f"xt{ec}", name=f"xt{ec}")
              for ec in range(EC)]
        for ec in range(EC):
            teng = (nc.sync, nc.gpsimd)[ec % 2]
            teng.dma_start(xt[ec][:, :], xt_ap[ec * 128:(ec + 1) * 128, :])
        wv = [w_pool.tile([128, E], BF16, tag=f"wv{ec}", name=f"wv{ec}")
              for ec in range(EC)]
        for ec in range(EC):
            nc.sync.dma_start(wv[ec][:, :], wv_ap[ec * 128:(ec + 1) * 128, :])
        # wosb is needed late (first out-proj chunk ~group 3): scalar's DMA
        # queue is idle until the first ACTIVATE, so it can issue these.
        wosb = [w_pool.tile([128, E], BF16, tag=f"wo{ec}", name=f"wo{ec}")
                for ec in range(EC)]
        for ec in range(EC):
            nc.scalar.dma_start(wosb[ec][:, :],
                                wout_ap[ec * 128:(ec + 1) * 128, :])

        # ---- constants (off the critical path) --------------------------
        bq = const_pool.tile([128, 6], FP32, tag="bq")
        nc.sync.dma_start(bq[:, :], bqkv_ap[0:E].rearrange(
            "(c p) -> p c", p=128)[:, 0:6])
        bv_row = const_pool.tile([1, E], FP32, tag="brow", name="bv_row")
        nc.gpsimd.dma_start(bv_row[:, :], bqkv_ap[2 * E:3 * E].unsqueeze(0))
        bv = const_pool.tile([128, E], FP32, tag="bv")
        nc.gpsimd.partition_broadcast(bv[:, :], bv_row[:, :], channels=128)
        bo_row = const_pool.tile([1, E], FP32, tag="brow", name="bo_row")
        nc.gpsimd.dma_start(bo_row[:, :], bout_ap.unsqueeze(0))
        bo = const_pool.tile([128, E], FP32, tag="bo")
        nc.gpsimd.partition_broadcast(bo[:, :], bo_row[:, :], channels=128)

        # ---- fp8 feature-major Q/K tiles (rows = pair's 2x64 d), 1 per fc
        qt8 = [qk8_pool.tile([128, T], FP8, tag=f"q8{f}", name=f"q8{f}")
               for f in range(6)]
        kt8 = [qk8_pool.tile([128, T], FP8, tag=f"k8{f}", name=f"k8{f}")
               for f in range(6)]

        def emit_qkproj(fc, tp):
            """One pse borrow: fp8-DR projection for fc over a pair of
            512-token windows, drained straight to the fp8 q/k tile."""
            ps = pse.tile([128, 1024], FP32, tag="pse")
            n0 = tp * 1024
            for cc in range(3):
                lhsT = w8[:, cc * 3072:(cc + 1) * 3072].rearrange(
                    "p (i f) -> p i f", i=2)[:, :, fc * 128:(fc + 1) * 128]
                rhs = x8[:, cc * 2 * T:(cc + 1) * 2 * T].rearrange(
                    "p (i n) -> p i n", i=2)
                for w in range(2):
                    nc.tensor.matmul(
                        ps[:, w * 512:(w + 1) * 512], lhsT,
                        rhs[:, :, n0 + w * 512:n0 + (w + 1) * 512],
                        start=(cc == 0), stop=(cc == 2), perf_mode=DR)
            if fc < 6:
                nc.vector.tensor_scalar_add(
                    qt8[fc][:, n0:n0 + 1024], ps[:, :], bq[:, fc:fc + 1])
            else:
                nc.vector.tensor_copy(kt8[fc - 6][:, n0:n0 + 1024], ps[:, :])

        # ---- V path -> VO (tok-major, ones col per head) ----------------
        vo = [vo_pool.tile([128, H * (D + 1)], BF16, tag=f"vo{i}",
                           name=f"vo{i}") for i in range(TC16)]

        def emit_v_chunk(tc16):
            ps = pse.tile([128, 1024], FP32, tag="pse")
            for ec in range(EC):
                nc.tensor.matmul(
                    ps[:, 0:512],
                    xt[ec][:, tc16 * 128:(tc16 + 1) * 128],
                    wv[ec][:, 0:512],
                    start=(ec == 0), stop=(ec == EC - 1))
            for ec in range(EC):
                nc.tensor.matmul(
                    ps[:, 512:768],
                    xt[ec][:, tc16 * 128:(tc16 + 1) * 128],
                    wv[ec][:, 512:768],
                    start=(ec == 0), stop=(ec == EC - 1))
            nc.vector.memset(vo[tc16][:, D::(D + 1)], 1.0)
            vo3a = vo[tc16][:, 0:8 * (D + 1)].rearrange(
                "p (h j) -> p h j", j=D + 1)[:, :, 0:D]
            nc.vector.tensor_tensor(
                vo3a, ps[:, 0:512].rearrange("p (h j) -> p h j", j=D),
                bv[:, 0:512].rearrange("p (h j) -> p h j", j=D), op=OP.add)
            vo3b = vo[tc16][:, 8 * (D + 1):].rearrange(
                "p (h j) -> p h j", j=D + 1)[:, :, 0:D]
            nc.vector.tensor_tensor(
                vo3b, ps[:, 512:768].rearrange("p (h j) -> p h j", j=D),
                bv[:, 512:768].rearrange("p (h j) -> p h j", j=D), op=OP.add)

        # ---- attention + out projection, software pipelined -------------
        et_pool = stack.enter_context(tc.tile_pool(name="et", bufs=10))
        small_pool = stack.enter_context(tc.tile_pool(name="small", bufs=1))
        rb_pool = stack.enter_context(tc.tile_pool(name="rb", bufs=1))
        osb_pool = stack.enter_context(tc.tile_pool(name="osb", bufs=2))

        dbl = [dbl_pool.tile([128, HM], BF16, tag=f"dbl{b}", name=f"dbl{b}")
               for b in range(BPC)]

        def emit_energy(fci, b, tq, tk, pe):
            for half in range(2):
                lo = 64 * half
                nc.tensor.matmul(
                    pe[:, half * 512:(half + 1) * 512],
                    kt8[fci][lo:lo + 64,
                             b * N + tk * 128:b * N + (tk + 1) * 128],
                    qt8[fci][lo:lo + 64,
                             b * N + tq * 512:b * N + (tq + 1) * 512],
                    start=True, stop=True)

        def emit_attnv_tk(st, pos, tk):
            b, fci, tq, ets = st
            for half in range(2):
                h = 2 * fci + half
                nc.tensor.matmul(
                    pos[:, half * 512:(half + 1) * 512],
                    vo[b * 8 + tk][:, h * (D + 1):(h + 1) * (D + 1)],
                    ets[tk][:, half * 512:(half + 1) * 512],
                    start=(tk == 0), stop=(tk == 7))

        def emit_drain(st, pos):
            b, fci, tq, _ = st
            sraw = small_pool.tile([1, 1024], FP32, tag="sraw")
            nc.vector.tensor_copy(sraw[:, :], pos[D:D + 1, :])
            rec = small_pool.tile([1, 1024], FP32, tag="rec")
            nc.vector.reciprocal_approx_fast(rec[:, :], sraw[:, :])
            rb = rb_pool.tile([64, 1024], FP32, tag="rb")
            nc.gpsimd.partition_broadcast(rb[:, :], rec[:, :], channels=64)
            for half in range(2):
                h = 2 * fci + half
                po = pos[:, half * 512:(half + 1) * 512]
                rbh = rb[:, half * 512:(half + 1) * 512]
                m0 = h * N + tq * 512
                nc.vector.tensor_tensor(
                    dbl[b][0:D, m0:m0 + 512], po[0:D, :], rbh, op=OP.mult)
                if m0 == 0:
                    nc.vector.tensor_tensor(
                        dbl[b][D:128, 0:511], po[0:D, 1:512], rbh[:, 1:512],
                        op=OP.mult)
                else:
                    nc.vector.tensor_tensor(
                        dbl[b][D:128, m0 - 1:m0 + 511], po[0:D, :], rbh,
                        op=OP.mult)

        def emit_outproj_chunk(b, npc):
            pf = pse.tile([128, 1024], FP32, tag="pse")
            for cc in range(EC):
                off = 2 * cc + 12 * (npc * 128)
                lhsT = dbl[b][:, off::12][:, 0:128]
                nc.tensor.matmul(pf[:, 0:512], lhsT, wosb[cc][:, 0:512],
                                 start=(cc == 0), stop=(cc == EC - 1))
            for cc in range(EC):
                off = 2 * cc + 12 * (npc * 128)
                lhsT = dbl[b][:, off::12][:, 0:128]
                nc.tensor.matmul(pf[:, 512:768], lhsT, wosb[cc][:, 512:768],
                                 start=(cc == 0), stop=(cc == EC - 1))
            osb = osb_pool.tile([128, E], FP32, tag="osb")
            nc.vector.tensor_tensor(osb[:, :], pf[:, 0:768], bo[:, :], op=OP.add)
            oeng = (nc.sync, nc.gpsimd)[npc % 2]
            oeng.dma_start(
                out_ap[b * N + npc * 128:b * N + (npc + 1) * 128, :], osb[:, :])

        # ---- pre-loop: batch-0 projections + first V chunks.  The extra
        # fc1/fc7 borrows fill the PE hole while xt/wv DMAs stream in.
        emit_qkproj(0, 0)
        emit_qkproj(6, 0)
        emit_qkproj(1, 0)
        emit_qkproj(7, 0)
        emit_v_chunk(0)
        emit_v_chunk(1)

        # ---- drip schedules ---------------------------------------------
        # qk-proj: pair fci reads the tp=0 (batch 0) window of fc=fci and
        # fc=6+fci at group 2*fci, and the tp=1 (batch 1) window only at
        # group 12; so all tp=0 borrows go first (2 per group), tp=1 after.
        qk_queue = []
        for fci in range(2, 6):
            for fc in (fci, 6 + fci):
                qk_queue.append((fc, 0))
        for fci in range(6):
            for fc in (fci, 6 + fci):
                qk_queue.append((fc, 1))
        # v chunks: b0 chunks 2..7 inside group 0 (first attn@V use is in
        # group 1 at the matching tk); b1 chunks needed from group 13.
        v_queue = list(range(8, 16))
        # out-proj: chunk npc needs scrambled m-prefix 1536*(npc+1)+1, i.e.
        # pair fci with 2048*(fci+1) >= 1536*(npc+1)+1; drain of (b,fci,tq1)
        # is emitted at the END of group b*12+2*fci+2.
        op_list = []   # (emit_group, b, npc)
        for b in range(BPC):
            done = 0
            for fci in range(6):
                hi = (2048 * (fci + 1) - 1 - 1536) // 1536  # max npc
                for npc in range(done, min(hi, 7) + 1):
                    op_list.append((b * 12 + 2 * fci + 3, b, npc))
                done = min(hi, 7) + 1
            for npc in range(done, 8):
                op_list.append((b * 12 + 13, b, npc))

        groups = [(b, fci, tq)
                  for b in range(BPC) for fci in range(6) for tq in range(2)]
        prev = None
        ops_now = []
        for gi, (b, fci, tq) in enumerate(groups):
            ops_now += [x for x in op_list if x[0] <= gi]
            op_list = [x for x in op_list if x[0] > gi]
            ets = []
            prev_pos = (pso.tile([65, 1024], FP32, tag="po", name="po")
                        if prev is not None else None)
            last = gi == len(groups) - 1
            for tk in range(8):
                pe = pse.tile([128, 1024], FP32, tag="pse")
                emit_energy(fci, b, tq, tk, pe)
                et = et_pool.tile([128, 1024], BF16, tag="et")
                nc.scalar.activation(et[:, :], pe[:, :], AF.Exp,
                                     bias=0.0, scale=SCALE)
                ets.append(et)
                if prev is not None:
                    if last:
                        # front-load the previous group's attn@V so its
                        # drain (gating the b1 npc6 chunk) runs early
                        if tk < 4:
                            emit_attnv_tk(prev, prev_pos, 2 * tk)
                            emit_attnv_tk(prev, prev_pos, 2 * tk + 1)
                        elif tk == 4:
                            emit_drain(prev, prev_pos)
                    else:
                        emit_attnv_tk(prev, prev_pos, tk)
                if gi == 0 and 2 <= tk <= 7:
                    emit_v_chunk(tk)   # vo[2..7] before group 1 reads them
                if tk == 1 and qk_queue and gi <= 10:
                    emit_qkproj(*qk_queue.pop(0))
                if tk == 3 and qk_queue and gi <= 10:
                    emit_qkproj(*qk_queue.pop(0))
                if tk == 5 and v_queue and 2 <= gi < 10:
                    emit_v_chunk(v_queue.pop(0))
                if tk in (2, 6) and ops_now:
                    emit_outproj_chunk(*ops_now.pop(0)[1:])
            if prev is not None and not last:
                emit_drain(prev, prev_pos)
            prev = (b, fci, tq, ets)
        prev_pos = pso.tile([65, 1024], FP32, tag="po", name="po")
        for tk in range(8):
            emit_attnv_tk(prev, prev_pos, tk)
            if tk in (2, 6) and ops_now:
                emit_outproj_chunk(*ops_now.pop(0)[1:])
        # everything except the last chunk (npc7) only needs drains already
        # emitted; the final drain gates only npc7.
        rest = sorted(ops_now + op_list)
        for x in [x for x in rest if x[2] != 7]:
            emit_outproj_chunk(*x[1:])
        emit_drain(prev, prev_pos)
        for x in [x for x in rest if x[2] == 7]:
            emit_outproj_chunk(*x[1:])


_built = None


def _build():
    global _built
    if _built is not None:
        return _built
    nc = bacc.Bacc("TRN2", target_bir_lowering=False, debug=False,
                   num_devices=N_CORES)
    x8_ap = nc.dram_tensor("x8", (128, 6 * T), FP8, kind="ExternalInput").ap()
    w8_ap = nc.dram_tensor("w8", (128, 9216), FP8, kind="ExternalInput").ap()
    xt_ap = nc.dram_tensor("xt", (E, T), BF16, kind="ExternalInput").ap()
    wv_ap = nc.dram_tensor("wv", (E, E), BF16, kind="ExternalInput").ap()
    bqkv_ap = nc.dram_tensor("b_qkv", (3 * E,), FP32, kind="ExternalInput").ap()
    wout_ap = nc.dram_tensor("w_out", (E, E), BF16, kind="ExternalInput").ap()
    bout_ap = nc.dram_tensor("b_out", (E,), FP32, kind="ExternalInput").ap()
    out_ap = nc.dram_tensor("out", (T, E), FP32, kind="ExternalOutput").ap()
    with tile.TileContext(nc) as tc:
        _emit(tc, x8_ap, w8_ap, xt_ap, wv_ap, bqkv_ap, wout_ap, bout_ap,
              out_ap)
    nc.compile()
    _built = nc
    return nc


def _prep_core(xT32):
    """Host-side per-core input prep from fp32 x^T (E, T)."""
    import ml_dtypes
    f8 = ml_dtypes.float8_e4m3
    xt = np.ascontiguousarray(xT32.astype(ml_dtypes.bfloat16))
    x8 = np.ascontiguousarray(
        xT32.reshape(3, 2, 128, T).transpose(2, 0, 1, 3)
        .reshape(128, 6 * T).astype(f8))
    return xt, x8


def kernel(x, W_qkv, b_qkv, W_out, b_out, _trace=False):
    import ml_dtypes
    f8 = ml_dtypes.float8_e4m3
    x = np.asarray(x, dtype=np.float32)
    xTs = [np.ascontiguousarray(
        x[c * BPC:(c + 1) * BPC].reshape(T, E).T) for c in range(N_CORES)]
    W_qkv = np.asarray(W_qkv, dtype=np.float32)
    w8 = np.ascontiguousarray(
        W_qkv[:, 0:2 * E].reshape(3, 2, 128, 2 * E).transpose(2, 0, 1, 3)
        .reshape(128, 9216).astype(f8))
    wv = np.ascontiguousarray(
        W_qkv[:, 2 * E:3 * E].astype(ml_dtypes.bfloat16))
    b_qkv = np.ascontiguousarray(np.asarray(b_qkv, dtype=np.float32))
    W_out = np.ascontiguousarray(
        np.asarray(W_out, dtype=np.float32).astype(ml_dtypes.bfloat16))
    b_out = np.ascontiguousarray(np.asarray(b_out, dtype=np.float32))

    nc = _build()
    in_maps = []
    for c in range(N_CORES):
        xt, x8 = _prep_core(xTs[c])
        in_maps.append({
            "x8": x8, "w8": w8, "xt": xt, "wv": wv,
            "b_qkv": b_qkv, "w_out": W_out, "b_out": b_out,
        })
    res = bass_utils.run_bass_kernel_spmd(
        nc, in_maps, core_ids=list(range(N_CORES)), trace=_trace)
    out = np.concatenate(
        [res.results[c]["out"].reshape(BPC, N, E) for c in range(N_CORES)],
        axis=0)
    if _trace:
        kernel._last_results = res
    return out


# revision 40
# speedup vs baseline: 1.1969x; 1.1969x over previous
"""Multi-head attention (B=16, N=1024, E=768, H=12) on 8 TRN2 NeuronCores.

Data parallel over batch (2 per core, no collectives). Per-core fused kernel
(fp8-DoubleRow Q/K projection + fused single-stream schedule):

  - Q/K projection in fp8-e4m3 DoubleRow (contraction 256 per matmul, so 3
    accumulation passes instead of 6): host passes x^T and W_qkv[:, :2E]
    pre-interleaved as [128, cc, i, n] with e = 256*cc + 128*i + p.
  - K-bias is dropped entirely: (q+bq)@(k+bk) = (q+bq)@k + f(q), and per-q
    terms cancel in softmax (exact).  Q gets its bias during the PSUM drain.
  - Q/K tiles stored fp8 feature-major (128 partitions = two heads' d-rows),
    12 tiles of (128, T); energy per head pair = two fp8 K=64 matmuls at PE
    row offsets 0/64 (row-tiled, partially concurrent) into one (128,1024)
    PSUM tile; one Exp ACTIVATE drains it (scale=1/8, no max subtraction -
    |logit| < 2 by construction).
  - attn@V in bf16: lhsT = [V | 1] (M=65); PSUM row 64 = softmax denom;
    both halves of a group accumulate into one (65,1024) PSUM tile so the
    denominator copy/reciprocal run once per group at (1,1024).
  - Normalization: reciprocal_approx_fast + gpsimd partition broadcast,
    fused into the PSUM->SBUF multiply that writes the shifted-duplicate
    "DOUBLE" layout; stride-12 APs over DOUBLE give exact 128-row slabs of
    Y^T for the reference's scrambled (H,N,D)->(N,E) reshape, so the out
    projection is 6 clean K=128 accumulating matmuls per 128-token tile.
  - Single fused schedule, batch-major: V-proj for batch 1, residual QK
    projection groups, and out-projection chunks drip through the attention
    loop's PE slack (borrowing PSUM slots); out-proj chunk npc only needs
    the scrambled m-prefix 1536*(npc+1)+1, so chunks start after ~1.5 heads
    of attention and only ~2 chunks remain after the last drain.
"""

import contextlib

import numpy as np

import concourse.bass as bass
import concourse.tile as tile
import concourse.mybir as mybir
from concourse import bacc
from concourse import bass_utils

B, N, E, H = 16, 1024, 768, 12
D = E // H          # 64
N_CORES = 8
BPC = B // N_CORES  # 2
T = BPC * N         # 2048
SCALE = 1.0 / float(np.sqrt(np.float32(D)))

FP32 = mybir.dt.float32
BF16 = mybir.dt.bfloat16
FP8 = mybir.dt.float8e4
AF = mybir.ActivationFunctionType
OP = mybir.AluOpType
DR = mybir.MatmulPerfMode.DoubleRow

EC = E // 128       # 6
TC16 = T // 128     # 16
HM = H * N          # 12288


def _emit(tc, x8_ap, w8_ap, xt_ap, wv_ap, bqkv_ap, wout_ap, bout_ap, out_ap):
    nc = tc.nc

    stack = contextlib.ExitStack()
    with stack:
        const_pool = stack.enter_context(tc.tile_pool(name="const", bufs=1))
        w_pool = stack.enter_context(tc.tile_pool(name="w", bufs=1))
        qk8_pool = stack.enter_context(tc.tile_pool(name="qk8", bufs=1))
        vo_pool = stack.enter_context(tc.tile_pool(name="vo", bufs=1))
        dbl_pool = stack.enter_context(tc.tile_pool(name="dbl", bufs=1))

        pse = stack.enter_context(
            tc.tile_pool(name="pse", bufs=2, space="PSUM"))   # (128,1024) = 2 banks
        pso = stack.enter_context(
            tc.tile_pool(name="pso", bufs=2, space="PSUM"))   # (65,1024) = 2 banks

        # ---- big input DMAs first (w8/x8 gate the first psum groups).
        # x8 is sliced by TOKEN half (strided over the i slots) so the
        # batch-0 projection's dependencies land first.
        x8 = w_pool.tile([128, 6 * T], FP8, tag="x8")
        w8 = w_pool.tile([128, 9216], FP8, tag="w8")
        for cc in range(3):
            nc.gpsimd.dma_start(w8[:, cc * 3072:(cc + 1) * 3072],
                                w8_ap[:, cc * 3072:(cc + 1) * 3072])
        for th in range(2):     # token half: 0:1024 first (batch 0)
            for cc in range(3):
                dst = x8[:, cc * 2 * T:(cc + 1) * 2 * T].rearrange(
                    "p (i n) -> p i n", i=2)[:, :, th * 1024:(th + 1) * 1024]
                src = x8_ap[:, cc * 2 * T:(cc + 1) * 2 * T].rearrange(
                    "p (i n) -> p i n", i=2)[:, :, th * 1024:(th + 1) * 1024]
                nc.sync.dma_start(dst, src)
        xt = [w_pool.tile([128, T], BF16, tag=f"xt{ec}", name=f"xt{ec}")
              for ec in range(EC)]
        for ec in range(EC):
            teng = (nc.sync, nc.gpsimd)[ec % 2]
            teng.dma_start(xt[ec][:, :], xt_ap[ec * 128:(ec + 1) * 128, :])
        wv = [w_pool.tile([128, E], BF16, tag=f"wv{ec}", name=f"wv{ec}")
              for ec in range(EC)]
        for ec in range(EC):
            nc.sync.dma_start(wv[ec][:, :], wv_ap[ec * 128:(ec + 1) * 128, :])
        # wosb is needed late (first out-proj chunk ~group 3): scalar's DMA
        # queue is idle until the first ACTIVATE, so it can issue these.
        wosb = [w_pool.tile([128, E], BF16, tag=f"wo{ec}", name=f"wo{ec}")
                for ec in range(EC)]
        for ec in range(EC):
            nc.scalar.dma_start(wosb[ec][:, :],
                                wout_ap[ec * 128:(ec + 1) * 128, :])

        # ---- constants (off the critical path) --------------------------
        bq = const_pool.tile([128, 6], FP32, tag="bq")
        nc.sync.dma_start(bq[:, :], bqkv_ap[0:E].rearrange(
            "(c p) -> p c", p=128)[:, 0:6])
        bv_row = const_pool.tile([1, E], FP32, tag="brow", name="bv_row")
        nc.gpsimd.dma_start(bv_row[:, :], bqkv_ap[2 * E:3 * E].unsqueeze(0))
        bv = const_pool.tile([128, E], FP32, tag="bv")
        nc.gpsimd.partition_broadcast(bv[:, :], bv_row[:, :], channels=128)
        bo_row = const_pool.tile([1, E], FP32, tag="brow", name="bo_row")
        nc.gpsimd.dma_start(bo_row[:, :], bout_ap.unsqueeze(0))
        bo = const_pool.tile([128, E], FP32, tag="bo")
        nc.gpsimd.partition_broadcast(bo[:, :], bo_row[:, :], channels=128)

        # ---- fp8 feature-major Q/K tiles (rows = pair's 2x64 d), 1 per fc
        qt8 = [qk8_pool.tile([128, T], FP8, tag=f"q8{f}", name=f"q8{f}")
               for f in range(6)]
        kt8 = [qk8_pool.tile([128, T], FP8, tag=f"k8{f}", name=f"k8{f}")
               for f in range(6)]

        def emit_qkproj(fc, tp):
            """One pse borrow: fp8-DR projection for fc over a pair of
            512-token windows, drained straight to the fp8 q/k tile."""
            ps = pse.tile([128, 1024], FP32, tag="pse")
            n0 = tp * 1024
            for cc in range(3):
                lhsT = w8[:, cc * 3072:(cc + 1) * 3072].rearrange(
                    "p (i f) -> p i f", i=2)[:, :, fc * 128:(fc + 1) * 128]
                rhs = x8[:, cc * 2 * T:(cc + 1) * 2 * T].rearrange(
                    "p (i n) -> p i n", i=2)
                for w in range(2):
                    nc.tensor.matmul(
                        ps[:, w * 512:(w + 1) * 512], lhsT,
                        rhs[:, :, n0 + w * 512:n0 + (w + 1) * 512],
                        start=(cc == 0), stop=(cc == 2), perf_mode=DR)
            if fc < 6:
                nc.vector.tensor_scalar_add(
                    qt8[fc][:, n0:n0 + 1024], ps[:, :], bq[:, fc:fc + 1])
            else:
                nc.vector.tensor_copy(kt8[fc - 6][:, n0:n0 + 1024], ps[:, :])

        # ---- V path -> VO (tok-major, ones col per head) ----------------
        vo = [vo_pool.tile([128, H * (D + 1)], BF16, tag=f"vo{i}",
                           name=f"vo{i}") for i in range(TC16)]

        def emit_v_chunk(tc16):
            ps = pse.tile([128, 1024], FP32, tag="pse")
            for ec in range(EC):
                nc.tensor.matmul(
                    ps[:, 0:512],
                    xt[ec][:, tc16 * 128:(tc16 + 1) * 128],
                    wv[ec][:, 0:512],
                    start=(ec == 0), stop=(ec == EC - 1))
            for ec in range(EC):
                nc.tensor.matmul(
                    ps[:, 512:768],
                    xt[ec][:, tc16 * 128:(tc16 + 1) * 128],
                    wv[ec][:, 512:768],
                    start=(ec == 0), stop=(ec == EC - 1))
            nc.vector.memset(vo[tc16][:, D::(D + 1)], 1.0)
            vo3a = vo[tc16][:, 0:8 * (D + 1)].rearrange(
                "p (h j) -> p h j", j=D + 1)[:, :, 0:D]
            nc.vector.tensor_tensor(
                vo3a, ps[:, 0:512].rearrange("p (h j) -> p h j", j=D),
                bv[:, 0:512].rearrange("p (h j) -> p h j", j=D), op=OP.add)
            vo3b = vo[tc16][:, 8 * (D + 1):].rearrange(
                "p (h j) -> p h j", j=D + 1)[:, :, 0:D]
            nc.vector.tensor_tensor(
                vo3b, ps[:, 512:768].rearrange("p (h j) -> p h j", j=D),
                bv[:, 512:768].rearrange("p (h j) -> p h j", j=D), op=OP.add)

        # ---- attention + out projection, software pipelined -------------
        et_pool = stack.enter_context(tc.tile_pool(name="et", bufs=10))
        small_pool = stack.enter_context(tc.tile_pool(name="small", bufs=1))
        rb_pool = stack.enter_context(tc.tile_pool(name="rb", bufs=1))
        osb_pool = stack.enter_context(tc.tile_pool(name="osb", bufs=2))

        dbl = [dbl_pool.tile([128, HM], BF16, tag=f"dbl{b}", name=f"dbl{b}")
               for b in range(BPC)]

        def emit_energy(fci, b, tq, tk, pe):
            for half in range(2):
                lo = 64 * half
                nc.tensor.matmul(
                    pe[:, half * 512:(half + 1) * 512],
                    kt8[fci][lo:lo + 64,
                             b * N + tk * 128:b * N + (tk + 1) * 128],
                    qt8[fci][lo:lo + 64,
                             b * N + tq * 512:b * N + (tq + 1) * 512],
                    start=True, stop=True)

        def emit_attnv_tk(st, pos, tk):
            b, fci, tq, ets = st
            for half in range(2):
                h = 2 * fci + half
                nc.tensor.matmul(
                    pos[:, half * 512:(half + 1) * 512],
                    vo[b * 8 + tk][:, h * (D + 1):(h + 1) * (D + 1)],
                    ets[tk][:, half * 512:(half + 1) * 512],
                    start=(tk == 0), stop=(tk == 7))

        def emit_drain(st, pos):
            b, fci, tq, _ = st
            sraw = small_pool.tile([1, 1024], FP32, tag="sraw")
            nc.vector.tensor_copy(sraw[:, :], pos[D:D + 1, :])
            rec = small_pool.tile([1, 1024], FP32, tag="rec")
            nc.vector.reciprocal_approx_fast(rec[:, :], sraw[:, :])
            rb = rb_pool.tile([64, 1024], FP32, tag="rb")
            nc.gpsimd.partition_broadcast(rb[:, :], rec[:, :], channels=64)
            for half in range(2):
                h = 2 * fci + half
                po = pos[:, half * 512:(half + 1) * 512]
                rbh = rb[:, half * 512:(half + 1) * 512]
                m0 = h * N + tq * 512
                nc.vector.tensor_tensor(
                    dbl[b][0:D, m0:m0 + 512], po[0:D, :], rbh, op=OP.mult)
                if m0 == 0:
                    nc.vector.tensor_tensor(
                        dbl[b][D:128, 0:511], po[0:D, 1:512], rbh[:, 1:512],
                        op=OP.mult)
                else:
                    nc.vector.tensor_tensor(
                        dbl[b][D:128, m0 - 1:m0 + 511], po[0:D, :], rbh,
                        op=OP.mult)

        def emit_outproj_chunk(b, npc):
            pf = pse.tile([128, 1024], FP32, tag="pse")
            for cc in range(EC):
                off = 2 * cc + 12 * (npc * 128)
                lhsT = dbl[b][:, off::12][:, 0:128]
                nc.tensor.matmul(pf[:, 0:512], lhsT, wosb[cc][:, 0:512],
                                 start=(cc == 0), stop=(cc == EC - 1))
            for cc in range(EC):
                off = 2 * cc + 12 * (npc * 128)
                lhsT = dbl[b][:, off::12][:, 0:128]
                nc.tensor.matmul(pf[:, 512:768], lhsT, wosb[cc][:, 512:768],
                                 start=(cc == 0), stop=(cc == EC - 1))
            osb = osb_pool.tile([128, E], FP32, tag="osb")
            nc.vector.tensor_tensor(osb[:, :], pf[:, 0:768], bo[:, :], op=OP.add)
            oeng = (nc.sync, nc.gpsimd)[npc % 2]
            oeng.dma_start(
                out_ap[b * N + npc * 128:b * N + (npc + 1) * 128, :], osb[:, :])

        # ---- pre-loop: batch-0 fc0/fc6 projections + first V chunks -----
        emit_qkproj(0, 0)
        emit_qkproj(6, 0)
        emit_v_chunk(0)
        emit_v_chunk(1)

        # ---- drip schedules ---------------------------------------------
        # qk-proj: pair fci reads the tp=0 (batch 0) window of fc=fci and
        # fc=6+fci at group 2*fci, and the tp=1 (batch 1) window only at
        # group 12; so all tp=0 borrows go first (2 per group), tp=1 after.
        qk_queue = []
        for fci in range(1, 6):
            for fc in (fci, 6 + fci):
                qk_queue.append((fc, 0))
        for fci in range(6):
            for fc in (fci, 6 + fci):
                qk_queue.append((fc, 1))
        # v chunks: b0 chunks 2..7 inside group 0 (first attn@V use is in
        # group 1 at the matching tk); b1 chunks needed from group 13.
        v_queue = list(range(8, 16))
        # out-proj: chunk npc needs scrambled m-prefix 1536*(npc+1)+1, i.e.
        # pair fci with 2048*(fci+1) >= 1536*(npc+1)+1; drain of (b,fci,tq1)
        # is emitted at the END of group b*12+2*fci+2.
        op_list = []   # (emit_group, b, npc)
        for b in range(BPC):
            done = 0
            for fci in range(6):
                hi = (2048 * (fci + 1) - 1 - 1536) // 1536  # max npc
                for npc in range(done, min(hi, 7) + 1):
                    op_list.append((b * 12 + 2 * fci + 3, b, npc))
                done = min(hi, 7) + 1
            for npc in range(done, 8):
                op_list.append((b * 12 + 13, b, npc))

        groups = [(b, fci, tq)
                  for b in range(BPC) for fci in range(6) for tq in range(2)]
        prev = None
        ops_now = []
        for gi, (b, fci, tq) in enumerate(groups):
            ops_now += [x for x in op_list if x[0] <= gi]
            op_list = [x for x in op_list if x[0] > gi]
            ets = []
            prev_pos = (pso.tile([65, 1024], FP32, tag="po", name="po")
                        if prev is not None else None)
            last = gi == len(groups) - 1
            for tk in range(8):
                pe = pse.tile([128, 1024], FP32, tag="pse")
                emit_energy(fci, b, tq, tk, pe)
                et = et_pool.tile([128, 1024], BF16, tag="et")
                nc.scalar.activation(et[:, :], pe[:, :], AF.Exp,
                                     bias=0.0, scale=SCALE)
                ets.append(et)
                if prev is not None:
                    if last:
                        # front-load the previous group's attn@V so its
                        # drain (gating the b1 npc6 chunk) runs early
                        if tk < 4:
                            emit_attnv_tk(prev, prev_pos, 2 * tk)
                            emit_attnv_tk(prev, prev_pos, 2 * tk + 1)
                        elif tk == 4:
                            emit_drain(prev, prev_pos)
                    else:
                        emit_attnv_tk(prev, prev_pos, tk)
                if gi == 0 and 2 <= tk <= 7:
                    emit_v_chunk(tk)   # vo[2..7] before group 1 reads them
                if tk == 1 and qk_queue and gi <= 10:
                    emit_qkproj(*qk_queue.pop(0))
                if tk == 3 and qk_queue and gi <= 10:
                    emit_qkproj(*qk_queue.pop(0))
                if tk == 5 and v_queue and 2 <= gi < 10:
                    emit_v_chunk(v_queue.pop(0))
                if tk in (2, 6) and ops_now:
                    emit_outproj_chunk(*ops_now.pop(0)[1:])
            if prev is not None and not last:
                emit_drain(prev, prev_pos)
            prev = (b, fci, tq, ets)
        prev_pos = pso.tile([65, 1024], FP32, tag="po", name="po")
        for tk in range(8):
            emit_attnv_tk(prev, prev_pos, tk)
            if tk in (2, 6) and ops_now:
                emit_outproj_chunk(*ops_now.pop(0)[1:])
        # everything except the last chunk (npc7) only needs drains already
        # emitted; the final drain gates only npc7.
        rest = sorted(ops_now + op_list)
        for x in [x for x in rest if x[2] != 7]:
            emit_outproj_chunk(*x[1:])
        emit_drain(prev, prev_pos)
        for x in [x for x in rest if x[2] == 7]:
            emit_outproj_chunk(*x[1:])


_built = None


def _build():
    global _built
    if _built is not None:
        return _built
    nc = bacc.Bacc("TRN2", target_bir_lowering=False, debug=False,
                   num_devices=N_CORES)
    x8_ap = nc.dram_tensor("x8", (128, 6 * T), FP8, kind="ExternalInput").ap()
    w8_ap = nc.dram_tensor("w8", (128, 9216), FP8, kind="ExternalInput").ap()
    xt_ap = nc.dram_tensor("xt", (E, T), BF16, kind="ExternalInput").ap()
    wv_ap = nc.dram_tensor("wv", (E, E), BF16, kind="ExternalInput").ap()
    bqkv_ap = nc.dram_tensor("b_qkv", (3 * E,), FP32, kind="ExternalInput").ap()
    wout_ap = nc.dram_tensor("w_out", (E, E), BF16, kind="ExternalInput").ap()
    bout_ap = nc.dram_tensor("b_out", (E,), FP32, kind="ExternalInput").ap()
    out_ap = nc.dram_tensor("out", (T, E), FP32, kind="ExternalOutput").ap()
    with tile.TileContext(nc) as tc:
        _emit(tc, x8_ap, w8_ap, xt_ap, wv_ap, bqkv_ap, wout_ap, bout_ap,
              out_ap)
    nc.compile()
    _built = nc
    return nc


def _prep_core(xT32):
    """Host-side per-core input prep from fp32 x^T (E, T)."""
    import ml_dtypes
    f8 = ml_dtypes.float8_e4m3
    xt = np.ascontiguousarray(xT32.astype(ml_dtypes.bfloat16))
    x8 = np.ascontiguousarray(
        xT32.reshape(3, 2, 128, T).transpose(2, 0, 1, 3)
        .reshape(128, 6 * T).astype(f8))
    return xt, x8


def kernel(x, W_qkv, b_qkv, W_out, b_out, _trace=False):
    import ml_dtypes
    f8 = ml_dtypes.float8_e4m3
    x = np.asarray(x, dtype=np.float32)
    xTs = [np.ascontiguousarray(
        x[c * BPC:(c + 1) * BPC].reshape(T, E).T) for c in range(N_CORES)]
    W_qkv = np.asarray(W_qkv, dtype=np.float32)
    w8 = np.ascontiguousarray(
        W_qkv[:, 0:2 * E].reshape(3, 2, 128, 2 * E).transpose(2, 0, 1, 3)
        .reshape(128, 9216).astype(f8))
    wv = np.ascontiguousarray(
        W_qkv[:, 2 * E:3 * E].astype(ml_dtypes.bfloat16))
    b_qkv = np.ascontiguousarray(np.asarray(b_qkv, dtype=np.float32))
    W_out = np.ascontiguousarray(
        np.asarray(W_out, dtype=np.float32).astype(ml_dtypes.bfloat16))
    b_out = np.ascontiguousarray(np.asarray(b_out, dtype=np.float32))

    nc = _build()
    in_maps = []
    for c in range(N_CORES):
        xt, x8 = _prep_core(xTs[c])
        in_maps.append({
            "x8": x8, "w8": w8, "xt": xt, "wv": wv,
            "b_qkv": b_qkv, "w_out": W_out, "b_out": b_out,
        })
    res = bass_utils.run_bass_kernel_spmd(
        nc, in_maps, core_ids=list(range(N_CORES)), trace=_trace)
    out = np.concatenate(
        [res.results[c]["out"].reshape(BPC, N, E) for c in range(N_CORES)],
        axis=0)
    if _trace:
        kernel._last_results = res
    return out
